# revision 7
# baseline (speedup 1.0000x reference)
"""DeepMemoryMachine Trainium2 Bass kernel (v3).

Model: 16384-step sequential GRU + discrete write-once memory:
    h_new = GRU(h_prev, x_t)
    q     = argmax(C_w @ h_new + C_b)          (512 addresses)
    hit (written[q] & q>0):   h_out = M[q]     (read replaces state)
    miss:                     h_out = h_new;  first-visit q>0 writes M[q]=h_new

v3 changes over v2 (measured: exec wall 11.6s -> ~2.1s; bass build
130s -> ~6s; rel err 2.1e-4, trajectory exact vs the fp32 reference):
* 1-core execution. The axon tunnel moves ~45 MB/s; v2 replicated the
  22 MB inputs + 16 MB outputs x8 = ~9.5s of pure transfer.
* Program-size diet (CH=128 + phase-1 in a For_i + fused output
  projection): the XLA/neuronx compile inside the timed exec call
  scales with program size (~3s for v2's ~12k instructions).
* Y is emitted as float16 (halves output + donated-zeros traffic;
  ~2.4e-4 relative error, far under the 2e-2 gate).
* Output projection (Y = H @ V_w.T + V_b) fused into the recurrence
  chunk loop as per-128-step PE matmul blocks - no Ht HBM round-trip,
  no separate phase 3.
* DMA-transposed loads (4-byte descriptors) replaced by contiguous
  loads + PE transpose (f32 is not xbar-transposable).
* PE kept HAM-warm (2.4 GHz): logits PSUM double-buffered so step
  t+1's C_b bias matmul issues while step t's argmax reads the other
  bank; a dummy matmul fills the argmax window. v2 left PE idle
  >3.4us/step -> 1.2 GHz for every matmul (13.7us/step measured).
* Gate chain shortened: sigmoid split (r first, z off critical path),
  h_new = ng*(1-z) + z*Mq with both multiplicands precomputed in the
  tanh shadow; C_b bias matmul as an fp16 hi/lo pair (2x512 f16 rows
  instead of 1x512 f32-at-4-cycles).

Precision: min top-2 logit gap along the trajectory is 9.3e-6 abs, so
every matmul feeding h or the logits stays full fp32 (one flipped
argmax diverges the trajectory). fp16 only where exact-ish (C_b hi/lo
split, error ~1e-8) or after the recurrence (Y output, ~2.4e-4).

Layout: 512-vectors are SBUF [128, 4] with element (p, j) = v[p + 128*j];
1536-vectors are [128, 12] likewise.  Gi = X @ W_ih.T + b_ih + b_hh(rz)
is precomputed on-device into HBM as GiT[12, 128, T], streamed per chunk.
"""

import numpy as np

import concourse.bass as bass
import concourse.bacc as bacc
import concourse.mybir as mybir
import concourse.tile as tile
from concourse import bass_utils
from concourse.masks import make_identity

F32 = mybir.dt.float32
F16 = mybir.dt.float16
U32 = mybir.dt.uint32
DVE = mybir.EngineType.DVE
ACT = mybir.EngineType.Activation
PE = mybir.EngineType.PE
SP = mybir.EngineType.SP
AF = mybir.ActivationFunctionType
OP = mybir.AluOpType

T_FULL = 16384
N_DIM = 256
M_DIM = 512     # hidden size; [128, 4] layout
KA = 512        # number of addresses (K+1)
L_OUT = 256
G3 = 3 * M_DIM  # 1536; [128, 12] layout


def build_nc(T=T_FULL, CH=128, loops=1, dummy_mm=1):
    YB = min(CH, 128)
    assert T % CH == 0 and CH % YB == 0 and T % 512 == 0
    NCH = T // CH
    nc = bacc.Bacc("TRN2", target_bir_lowering=False, debug=False,
                   enable_asserts=False)

    X_d = nc.dram_tensor("X", [T, N_DIM], F32, kind="ExternalInput")
    h0_d = nc.dram_tensor("h0", [M_DIM], F32, kind="ExternalInput")
    Wih_d = nc.dram_tensor("W_ih", [G3, N_DIM], F32, kind="ExternalInput")
    Whh_d = nc.dram_tensor("W_hh", [G3, M_DIM], F32, kind="ExternalInput")
    bih_d = nc.dram_tensor("b_ih", [G3], F32, kind="ExternalInput")
    bhh_d = nc.dram_tensor("b_hh", [G3], F32, kind="ExternalInput")
    Cw_d = nc.dram_tensor("C_w", [KA, M_DIM], F32, kind="ExternalInput")
    Cb_d = nc.dram_tensor("C_b", [KA], F32, kind="ExternalInput")
    Vw_d = nc.dram_tensor("V_w", [L_OUT, M_DIM], F32, kind="ExternalInput")
    Vb_d = nc.dram_tensor("V_b", [L_OUT], F32, kind="ExternalInput")
    Y_d = nc.dram_tensor("Y", [T, L_OUT], F16, kind="ExternalOutput")

    GiT_d = nc.dram_tensor("GiT", [12, 128, T], F32, kind="Internal")

    with tile.TileContext(nc) as tc:
        with (
            tc.tile_pool(name="state", bufs=1) as st,
            tc.tile_pool(name="scratch", bufs=1) as sc,
            tc.tile_pool(name="dbuf", bufs=2) as db,
            tc.tile_pool(name="psA", bufs=1, space="PSUM") as ppA,
            tc.tile_pool(name="psB", bufs=2, space="PSUM") as ppB,
            tc.tile_pool(name="psL", bufs=2, space="PSUM") as ppL,
            tc.tile_pool(name="psY", bufs=1, space="PSUM") as ppY,
        ):
            # ---- persistent weights / state in SBUF ----
            WihT = st.tile([128, 24, 128], F32)      # (kp, k*12+j, m)
            WhhT = st.tile([128, 48, 128], F32)      # (dp, k*12+j, m)
            CwT = st.tile([128, 4, KA], F32)         # (dp, k, a)
            VwT = st.tile([128, 4, L_OUT], F32)      # (dp, k, l)
            bb_row = st.tile([1, G3], F32)           # b_ih + b_hh (r,z only)
            bhh_row = st.tile([1, G3], F32)
            bhh_nT = st.tile([1, M_DIM], F32)        # b_hn row (for GHtab)
            Cb_hi = st.tile([1, KA], F16)
            Cb_lo = st.tile([1, KA], F16)
            Vb_row = st.tile([1, L_OUT], F32)
            ones_row = st.tile([1, 512], F32)
            one4 = st.tile([128, 4], F32)
            one1 = st.tile([1, 1], F32)
            one1h = st.tile([1, 1], F16)
            POS = st.tile([1, KA], U32)              # [0,1,1,...,1]
            ident = st.tile([128, 128], F32)         # PE transpose identity

            GHtab = st.tile([128, 12, KA], F32)      # memoized W_hh@M[q] + bhn
            Mtab = st.tile([128, 4, KA], F32)        # memory rows
            wflag = st.tile([1, KA], U32)            # written flags
            Hbuf = st.tile([128, CH + 1, 4], F32)    # h_out history (chunk)
            giT = st.tile([128, 12, CH], F32)        # streamed Gi chunk
            idx8 = st.tile([1, 8], U32)
            mx8 = st.tile([1, 8], F32)

            g4r = sc.tile([128, 4], F32)
            g4z = sc.tile([128, 4], F32)
            rr = sc.tile([128, 4], F32)
            zz = sc.tile([128, 4], F32)
            omz = sc.tile([128, 4], F32)
            zm = sc.tile([128, 4], F32)
            t4 = sc.tile([128, 4], F32)
            u4 = sc.tile([128, 4], F32)
            ng = sc.tile([128, 4], F32)
            e4 = sc.tile([128, 4], F32)
            hnew = sc.tile([128, 4], F32)

            gh_ps = ppA.tile([128, 12], F32)
            dummy_ps = ppA.tile([1, 512], F32)

            # ---- constants ----
            nc.vector.memset(ones_row[:], 1.0)
            nc.vector.memset(one4[:], 1.0)
            nc.vector.memset(one1[:], 1.0)
            nc.vector.memset(one1h[:], 1.0)
            nc.vector.memset(wflag[:], 0)
            nc.vector.memset(POS[:], 1)
            nc.vector.memset(POS[0:1, 0:1], 0)
            nc.vector.memset(idx8[:], 0)
            nc.vector.memset(Mtab[:], 0.0)
            nc.vector.memset(GHtab[:], 0.0)
            nc.vector.memset(Hbuf[:], 0.0)
            make_identity(nc, ident[:])

            # ---- one-time weight loads: contiguous DMA + PE transpose ----
            def load_T(dst_ap, src_ap, tag):
                """dst[128,128] (SBUF) = src[128,128] (DRAM).T via PE."""
                raw = db.tile([128, 128], F32, tag=f"ldT_{tag}")
                nc.sync.dma_start(raw[:], src_ap)
                tp = ppB.tile([128, 128], F32, tag="ps")
                nc.tensor.transpose(tp[:], raw[:], ident[:])
                nc.vector.tensor_copy(dst_ap, tp[:])

            for j in range(12):
                for k in range(2):
                    load_T(WihT[:, k * 12 + j, :],
                           Wih_d[128 * j:128 * (j + 1),
                                 128 * k:128 * (k + 1)], "w")
                for k in range(4):
                    load_T(WhhT[:, k * 12 + j, :],
                           Whh_d[128 * j:128 * (j + 1),
                                 128 * k:128 * (k + 1)], "w")
            for k in range(4):
                for a in range(4):
                    load_T(CwT[:, k, 128 * a:128 * (a + 1)],
                           Cw_d[128 * a:128 * (a + 1),
                                128 * k:128 * (k + 1)], "w")
                for a in range(2):
                    load_T(VwT[:, k, 128 * a:128 * (a + 1)],
                           Vw_d[128 * a:128 * (a + 1),
                                128 * k:128 * (k + 1)], "w")

            nc.sync.dma_start(bb_row[:], bih_d[None, :])
            nc.sync.dma_start(bhh_row[:], bhh_d[None, :])
            nc.sync.dma_start(Vb_row[:], Vb_d[None, :])
            nc.sync.dma_start(bhh_nT[:], bhh_d[None, 2 * M_DIM:3 * M_DIM])
            # fold b_hh into Gi for the r,z gates only; b_hn lives in GHtab
            # (torch GRU: n = tanh(W_in x + b_in + r*(W_hn h + b_hn)))
            nc.vector.tensor_tensor(bb_row[0:1, 0:2 * M_DIM],
                                    bb_row[0:1, 0:2 * M_DIM],
                                    bhh_row[0:1, 0:2 * M_DIM], OP.add)

            # C_b as an fp16 hi/lo pair (residual error ~1e-8)
            Cb_f32 = sc.tile([1, KA], F32, tag="cbtmp")
            Cb_rest = sc.tile([1, KA], F32, tag="cbtmp2")
            nc.sync.dma_start(Cb_f32[:], Cb_d[None, :])
            nc.vector.tensor_copy(Cb_hi[:], Cb_f32[:])       # f32 -> f16 round
            nc.vector.tensor_copy(Cb_rest[:], Cb_hi[:])      # f16 -> f32 exact
            nc.vector.tensor_tensor(Cb_rest[:], Cb_f32[:], Cb_rest[:],
                                    OP.subtract)
            nc.vector.tensor_copy(Cb_lo[:], Cb_rest[:])      # residual -> f16

            # h_prev is always read from Mtab[q]; q starts at 0
            nc.sync.dma_start(Mtab[:, :, 0],
                              h0_d.rearrange("(j p) -> p j", p=128))
            h0_sb = sc.tile([128, 4], F32, tag="h0sb")
            nc.sync.dma_start(h0_sb[:], h0_d.rearrange("(j p) -> p j", p=128))

            def gh_matmuls(hsrc):
                """gh_ps[:, j] = (W_hh @ h)[128j:128j+128], + b_hn on j>=8."""
                for j in range(12):
                    for k in range(4):
                        nc.tensor.matmul(gh_ps[:, j:j + 1],
                                         WhhT[:, k * 12 + j, :],
                                         hsrc[:, k:k + 1],
                                         start=(k == 0),
                                         stop=(k == 3 and j < 8))
                    if j >= 8:
                        nc.tensor.matmul(gh_ps[:, j:j + 1],
                                         bhh_nT[0:1, 128 * (j - 8):128 * (j - 7)],
                                         one1[:], start=False, stop=True)

            gh_matmuls(h0_sb)
            nc.vector.tensor_copy(GHtab[:, :, 0], gh_ps[:])

            # ---- phase 1: GiT = (X @ W_ih.T + bb) as [12,128,T] ----
            P1C = 512
            with tc.For_i(0, T // P1C) as c1:
                xts = []
                for k in range(2):
                    xt = db.tile([128, P1C], F32, tag=f"xt{k}")
                    xts.append(xt)
                for b in range(4):
                    raw = db.tile([128, N_DIM], F32, tag="xraw")
                    nc.sync.dma_start(
                        raw[:], X_d[bass.ds(c1 * P1C + b * 128, 128), :])
                    for k in range(2):
                        tp = ppB.tile([128, 128], F32, tag="ps")
                        nc.tensor.transpose(
                            tp[:], raw[:, 128 * k:128 * (k + 1)], ident[:])
                        nc.vector.tensor_copy(
                            xts[k][:, 128 * b:128 * (b + 1)], tp[:])
                for j in range(12):
                    ps = ppB.tile([128, P1C], F32, tag="ps")
                    nc.tensor.matmul(ps[:], bb_row[0:1, 128 * j:128 * (j + 1)],
                                     ones_row[0:1, 0:P1C],
                                     start=True, stop=False)
                    for k in range(2):
                        nc.tensor.matmul(ps[:], WihT[:, k * 12 + j, :],
                                         xts[k][:], start=False, stop=(k == 1))
                    gi_out = db.tile([128, P1C], F32, tag="giout")
                    nc.scalar.activation(gi_out[:], ps[:], AF.Copy)
                    nc.sync.dma_start(GiT_d[j, :, bass.ds(c1 * P1C, P1C)],
                                      gi_out[:])

            # ---- phase 2: the recurrence (+ fused output projection) ----
            Gi_v = GiT_d.rearrange("j p t -> p j t")

            def load_q(engines):
                return nc.values_load(
                    idx8[0:1, 0:1], engines=engines,
                    min_val=0, max_val=KA - 1,
                    skip_runtime_bounds_check=True)

            with tc.For_i(0, NCH * loops,
                          hint_engines=(PE, DVE, ACT, SP)) as ch_raw:
                ch = (ch_raw % NCH) if loops > 1 else ch_raw
                nc.sync.dma_start(giT[:], Gi_v[:, :, bass.ds(ch * CH, CH)])
                q = load_q([DVE, ACT, PE])

                for i in range(CH):
                    lg = ppL.tile([1, KA], F32, tag="lg")
                    # C_b bias (fp16 hi/lo pair) issues before gates finish;
                    # double-buffered PSUM lets it overlap prev step's argmax
                    nc.tensor.matmul(lg[:], one1h[:], Cb_hi[:],
                                     start=True, stop=False)
                    nc.tensor.matmul(lg[:], one1h[:], Cb_lo[:],
                                     start=False, stop=False)
                    if dummy_mm >= 2:
                        # fills the PE stall while the gates compute h_new
                        nc.tensor.matmul(dummy_ps[:], one1[:], ones_row[:],
                                         start=True, stop=True)
                    # gates: r on the critical path, z in its shadow
                    nc.vector.tensor_tensor(
                        g4r[:], giT[:, 0:4, bass.ds(i, 1)],
                        GHtab[:, 0:4, bass.ds(q, 1)], OP.add)
                    nc.vector.tensor_tensor(
                        g4z[:], giT[:, 4:8, bass.ds(i, 1)],
                        GHtab[:, 4:8, bass.ds(q, 1)], OP.add)
                    nc.scalar.activation(rr[:], g4r[:], AF.Sigmoid)
                    nc.vector.tensor_tensor(t4[:], rr[:],
                                            GHtab[:, 8:12, bass.ds(q, 1)],
                                            OP.mult)
                    nc.vector.tensor_tensor(u4[:], t4[:],
                                            giT[:, 8:12, bass.ds(i, 1)], OP.add)
                    nc.scalar.activation(zz[:], g4z[:], AF.Sigmoid)
                    nc.vector.tensor_tensor(omz[:], one4[:], zz[:], OP.subtract)
                    nc.vector.tensor_tensor(zm[:], zz[:],
                                            Mtab[:, :, bass.ds(q, 1)], OP.mult)
                    nc.scalar.activation(ng[:], u4[:], AF.Tanh)
                    # h_new = ng*(1-z) + z*h_prev
                    nc.vector.tensor_tensor(e4[:], ng[:], omz[:], OP.mult)
                    nc.vector.tensor_tensor(hnew[:], e4[:], zm[:], OP.add)
                    # logits += C_w @ h_new
                    for k in range(4):
                        nc.tensor.matmul(lg[:], hnew[:, k:k + 1],
                                         CwT[:, k, :],
                                         start=False, stop=(k == 3))
                    if dummy_mm >= 1:
                        # HAM warm-keeper: runs during the argmax window
                        nc.tensor.matmul(dummy_ps[:], one1[:], ones_row[:],
                                         start=True, stop=True)
                    if dummy_mm >= 3:
                        nc.tensor.matmul(dummy_ps[:], one1[:], ones_row[:],
                                         start=True, stop=True)
                    # argmax straight from PSUM
                    nc.vector.max(mx8[:], lg[:])
                    nc.vector.max_index(idx8[:], mx8[:], lg[:])
                    q2 = load_q([DVE, ACT, PE])
                    f = nc.values_load(wflag[0:1, bass.ds(q2, 1)],
                                       engines=[ACT, PE],
                                       skip_runtime_bounds_check=True)
                    with tc.If(f == 0):
                        # miss: write-once M/GHtab update (ACT+PE only)
                        nc.scalar.copy(Mtab[:, :, bass.ds(q2, 1)], hnew[:])
                        nc.scalar.copy(wflag[0:1, bass.ds(q2, 1)],
                                       POS[0:1, bass.ds(q2, 1)])
                        gh_matmuls(hnew)
                        nc.scalar.copy(GHtab[:, :, bass.ds(q2, 1)], gh_ps[:])
                    # h_out == Mtab[q2] on both paths (miss wrote it first)
                    nc.scalar.copy(Hbuf[:, i + 1, :],
                                   Mtab[:, :, bass.ds(q2, 1)])
                    q = q2

                    if (i + 1) % YB == 0:
                        # fused output projection for the last YB steps
                        tt = i // YB
                        ps_y = ppY.tile([YB, L_OUT], F32, tag="psy")
                        nc.tensor.matmul(ps_y[:], ones_row[0:1, 0:YB],
                                         Vb_row[:], start=True, stop=False)
                        for j in range(4):
                            nc.tensor.matmul(
                                ps_y[:],
                                Hbuf[:, 1 + tt * YB:1 + (tt + 1) * YB, j],
                                VwT[:, j, :], start=False, stop=(j == 3))
                        y16 = db.tile([YB, L_OUT], F16, tag="y16")
                        nc.vector.tensor_copy(y16[:], ps_y[:])
                        nc.sync.dma_start(
                            Y_d[bass.ds(ch * CH + tt * YB, YB), :], y16[:])

            if dummy_mm:
                # anchor so the warm-keeper matmuls aren't dead-code-eliminated
                junk = sc.tile([1, 512], F32, tag="junk")
                nc.vector.tensor_copy(junk[:], dummy_ps[:])
                nc.sync.dma_start(GiT_d[0, 0:1, 0:512], junk[:])

    nc.compile()
    return nc


_NC_CACHE = {}


def _get_nc(T=T_FULL, CH=128):
    key = (T, CH)
    if key not in _NC_CACHE:
        _NC_CACHE[key] = build_nc(T, CH)
    return _NC_CACHE[key]


def kernel(**inputs):
    nc = _get_nc()
    in_map = {k: np.ascontiguousarray(np.asarray(v, np.float32))
              for k, v in inputs.items()}
    res = bass_utils.run_bass_kernel_spmd(nc, [in_map], core_ids=[0])
    return res.results[0]["Y"].astype(np.float32)


# revision 11
# speedup vs baseline: 1.1550x; 1.1550x over previous
"""DeepMemoryMachine Trainium2 Bass kernel (v3).

Model: 16384-step sequential GRU + discrete write-once memory:
    h_new = GRU(h_prev, x_t)
    q     = argmax(C_w @ h_new + C_b)          (512 addresses)
    hit (written[q] & q>0):   h_out = M[q]     (read replaces state)
    miss:                     h_out = h_new;  first-visit q>0 writes M[q]=h_new

v3 changes over v2 (measured: exec wall 11.6s -> ~2.1s; bass build
130s -> ~6s; rel err 2.1e-4, trajectory exact vs the fp32 reference):
* 1-core execution. The axon tunnel moves ~45 MB/s; v2 replicated the
  22 MB inputs + 16 MB outputs x8 = ~9.5s of pure transfer.
* Program-size diet (CH=128 + phase-1 in a For_i + fused output
  projection): the XLA/neuronx compile inside the timed exec call
  scales with program size (~3s for v2's ~12k instructions).
* Y is emitted as float16 (halves output + donated-zeros traffic;
  ~2.4e-4 relative error, far under the 2e-2 gate).
* Output projection (Y = H @ V_w.T + V_b) fused into the recurrence
  chunk loop as per-128-step PE matmul blocks - no Ht HBM round-trip,
  no separate phase 3.
* DMA-transposed loads (4-byte descriptors) replaced by contiguous
  loads + PE transpose (f32 is not xbar-transposable).
* PE kept HAM-warm (2.4 GHz): logits PSUM double-buffered so step
  t+1's C_b bias matmul issues while step t's argmax reads the other
  bank; a dummy matmul fills the argmax window. v2 left PE idle
  >3.4us/step -> 1.2 GHz for every matmul (13.7us/step measured).
* Gate chain shortened: sigmoid split (r first, z off critical path),
  h_new = ng*(1-z) + z*Mq with both multiplicands precomputed in the
  tanh shadow; C_b bias matmul as an fp16 hi/lo pair (2x512 f16 rows
  instead of 1x512 f32-at-4-cycles).

Precision: min top-2 logit gap along the trajectory is 9.3e-6 abs, so
every matmul feeding h or the logits stays full fp32 (one flipped
argmax diverges the trajectory). fp16 only where exact-ish (C_b hi/lo
split, error ~1e-8) or after the recurrence (Y output, ~2.4e-4).

Layout: 512-vectors are SBUF [128, 4] with element (p, j) = v[p + 128*j];
1536-vectors are [128, 12] likewise.  Gi = X @ W_ih.T + b_ih + b_hh(rz)
is precomputed on-device into HBM as GiT[12, 128, T], streamed per chunk.
"""

import numpy as np

import concourse.bass as bass
import concourse.bacc as bacc
import concourse.mybir as mybir
import concourse.tile as tile
from concourse import bass_utils
from concourse.masks import make_identity

F32 = mybir.dt.float32
F16 = mybir.dt.float16
U32 = mybir.dt.uint32
DVE = mybir.EngineType.DVE
ACT = mybir.EngineType.Activation
PE = mybir.EngineType.PE
SP = mybir.EngineType.SP
AF = mybir.ActivationFunctionType
OP = mybir.AluOpType

T_FULL = 16384
N_DIM = 256
M_DIM = 512     # hidden size; [128, 4] layout
KA = 512        # number of addresses (K+1)
L_OUT = 256
G3 = 3 * M_DIM  # 1536; [128, 12] layout


def build_nc(T=T_FULL, CH=128, loops=1, dummy_mm=1):
    YB = min(CH, 128)
    assert T % CH == 0 and CH % YB == 0 and T % 512 == 0
    NCH = T // CH
    nc = bacc.Bacc("TRN2", target_bir_lowering=False, debug=False,
                   enable_asserts=False)

    X_d = nc.dram_tensor("X", [T, N_DIM], F32, kind="ExternalInput")
    h0_d = nc.dram_tensor("h0", [M_DIM], F32, kind="ExternalInput")
    Wih_d = nc.dram_tensor("W_ih", [G3, N_DIM], F32, kind="ExternalInput")
    Whh_d = nc.dram_tensor("W_hh", [G3, M_DIM], F32, kind="ExternalInput")
    bih_d = nc.dram_tensor("b_ih", [G3], F32, kind="ExternalInput")
    bhh_d = nc.dram_tensor("b_hh", [G3], F32, kind="ExternalInput")
    Cw_d = nc.dram_tensor("C_w", [KA, M_DIM], F32, kind="ExternalInput")
    Cb_d = nc.dram_tensor("C_b", [KA], F32, kind="ExternalInput")
    Vw_d = nc.dram_tensor("V_w", [L_OUT, M_DIM], F32, kind="ExternalInput")
    Vb_d = nc.dram_tensor("V_b", [L_OUT], F32, kind="ExternalInput")
    Y_d = nc.dram_tensor("Y", [T, L_OUT], F16, kind="ExternalOutput")

    GiT_d = nc.dram_tensor("GiT", [12, 128, T], F32, kind="Internal")

    with tile.TileContext(nc) as tc:
        with (
            tc.tile_pool(name="state", bufs=1) as st,
            tc.tile_pool(name="scratch", bufs=1) as sc,
            tc.tile_pool(name="dbuf", bufs=2) as db,
            tc.tile_pool(name="psA", bufs=1, space="PSUM") as ppA,
            tc.tile_pool(name="psB", bufs=2, space="PSUM") as ppB,
            tc.tile_pool(name="psL", bufs=2, space="PSUM") as ppL,
            tc.tile_pool(name="psY", bufs=1, space="PSUM") as ppY,
        ):
            # ---- persistent weights / state in SBUF ----
            WihT = st.tile([128, 24, 128], F32)      # (kp, k*12+j, m)
            WhhT = st.tile([128, 48, 128], F32)      # (dp, k*12+j, m)
            CwT = st.tile([128, 4, KA], F32)         # (dp, k, a)
            VwT = st.tile([128, 4, L_OUT], F32)      # (dp, k, l)
            bb_row = st.tile([1, G3], F32)           # b_ih + b_hh (r,z only)
            bhh_row = st.tile([1, G3], F32)
            bhh_nT = st.tile([1, M_DIM], F32)        # b_hn row (for GHtab)
            Cb_hi = st.tile([1, KA], F16)
            Cb_lo = st.tile([1, KA], F16)
            Vb_row = st.tile([1, L_OUT], F32)
            ones_row = st.tile([1, 512], F32)
            one4 = st.tile([128, 4], F32)
            one1 = st.tile([1, 1], F32)
            one1h = st.tile([1, 1], F16)
            POS = st.tile([1, KA], U32)              # [0,1,1,...,1]
            ident = st.tile([128, 128], F32)         # PE transpose identity

            GHtab = st.tile([128, 12, KA], F32)      # memoized W_hh@M[q] + bhn
            Mtab = st.tile([128, 4, KA], F32)        # memory rows
            wflag = st.tile([1, KA], U32)            # written flags
            Hbuf = st.tile([128, CH + 1, 4], F32)    # h_out history (chunk)
            giT = st.tile([128, 12, CH], F32)        # streamed Gi chunk
            idx8 = st.tile([1, 8], U32)
            mx8 = st.tile([1, 8], F32)

            g4r = sc.tile([128, 4], F32)
            g4z = sc.tile([128, 4], F32)
            rr = sc.tile([128, 4], F32)
            zz = sc.tile([128, 4], F32)
            omz = sc.tile([128, 4], F32)
            zm = sc.tile([128, 4], F32)
            t4 = sc.tile([128, 4], F32)
            u4 = sc.tile([128, 4], F32)
            ng = sc.tile([128, 4], F32)
            e4 = sc.tile([128, 4], F32)
            hnew = sc.tile([128, 4], F32)

            gh_ps = ppA.tile([128, 12], F32)
            dummy_ps = ppA.tile([1, 512], F32)

            # ---- constants ----
            nc.vector.memset(ones_row[:], 1.0)
            nc.vector.memset(one4[:], 1.0)
            nc.vector.memset(one1[:], 1.0)
            nc.vector.memset(one1h[:], 1.0)
            nc.vector.memset(wflag[:], 0)
            nc.vector.memset(POS[:], 1)
            nc.vector.memset(POS[0:1, 0:1], 0)
            nc.vector.memset(idx8[:], 0)
            nc.vector.memset(Mtab[:], 0.0)
            nc.vector.memset(GHtab[:], 0.0)
            nc.vector.memset(Hbuf[:], 0.0)
            make_identity(nc, ident[:])

            # ---- one-time weight loads: contiguous DMA + PE transpose ----
            def load_T(dst_ap, src_ap, tag):
                """dst[128,128] (SBUF) = src[128,128] (DRAM).T via PE."""
                raw = db.tile([128, 128], F32, tag=f"ldT_{tag}")
                nc.sync.dma_start(raw[:], src_ap)
                tp = ppB.tile([128, 128], F32, tag="ps")
                nc.tensor.transpose(tp[:], raw[:], ident[:])
                nc.vector.tensor_copy(dst_ap, tp[:])

            for j in range(12):
                for k in range(2):
                    load_T(WihT[:, k * 12 + j, :],
                           Wih_d[128 * j:128 * (j + 1),
                                 128 * k:128 * (k + 1)], "w")
                for k in range(4):
                    load_T(WhhT[:, k * 12 + j, :],
                           Whh_d[128 * j:128 * (j + 1),
                                 128 * k:128 * (k + 1)], "w")
            for k in range(4):
                for a in range(4):
                    load_T(CwT[:, k, 128 * a:128 * (a + 1)],
                           Cw_d[128 * a:128 * (a + 1),
                                128 * k:128 * (k + 1)], "w")
                for a in range(2):
                    load_T(VwT[:, k, 128 * a:128 * (a + 1)],
                           Vw_d[128 * a:128 * (a + 1),
                                128 * k:128 * (k + 1)], "w")

            nc.sync.dma_start(bb_row[:], bih_d[None, :])
            nc.sync.dma_start(bhh_row[:], bhh_d[None, :])
            nc.sync.dma_start(Vb_row[:], Vb_d[None, :])
            nc.sync.dma_start(bhh_nT[:], bhh_d[None, 2 * M_DIM:3 * M_DIM])
            # fold b_hh into Gi for the r,z gates only; b_hn lives in GHtab
            # (torch GRU: n = tanh(W_in x + b_in + r*(W_hn h + b_hn)))
            nc.vector.tensor_tensor(bb_row[0:1, 0:2 * M_DIM],
                                    bb_row[0:1, 0:2 * M_DIM],
                                    bhh_row[0:1, 0:2 * M_DIM], OP.add)

            # C_b as an fp16 hi/lo pair (residual error ~1e-8)
            Cb_f32 = sc.tile([1, KA], F32, tag="cbtmp")
            Cb_rest = sc.tile([1, KA], F32, tag="cbtmp2")
            nc.sync.dma_start(Cb_f32[:], Cb_d[None, :])
            nc.vector.tensor_copy(Cb_hi[:], Cb_f32[:])       # f32 -> f16 round
            nc.vector.tensor_copy(Cb_rest[:], Cb_hi[:])      # f16 -> f32 exact
            nc.vector.tensor_tensor(Cb_rest[:], Cb_f32[:], Cb_rest[:],
                                    OP.subtract)
            nc.vector.tensor_copy(Cb_lo[:], Cb_rest[:])      # residual -> f16

            # h_prev is always read from Mtab[q]; q starts at 0
            nc.sync.dma_start(Mtab[:, :, 0],
                              h0_d.rearrange("(j p) -> p j", p=128))
            h0_sb = sc.tile([128, 4], F32, tag="h0sb")
            nc.sync.dma_start(h0_sb[:], h0_d.rearrange("(j p) -> p j", p=128))

            def gh_matmuls(hsrc):
                """gh_ps[:, j] = (W_hh @ h)[128j:128j+128], + b_hn on j>=8."""
                for j in range(12):
                    for k in range(4):
                        nc.tensor.matmul(gh_ps[:, j:j + 1],
                                         WhhT[:, k * 12 + j, :],
                                         hsrc[:, k:k + 1],
                                         start=(k == 0),
                                         stop=(k == 3 and j < 8))
                    if j >= 8:
                        nc.tensor.matmul(gh_ps[:, j:j + 1],
                                         bhh_nT[0:1, 128 * (j - 8):128 * (j - 7)],
                                         one1[:], start=False, stop=True)

            gh_matmuls(h0_sb)
            nc.vector.tensor_copy(GHtab[:, :, 0], gh_ps[:])

            # ---- phase 1: GiT = (X @ W_ih.T + bb) as [12,128,T] ----
            P1C = 512
            with tc.For_i(0, T // P1C) as c1:
                xts = []
                for k in range(2):
                    xt = db.tile([128, P1C], F32, tag=f"xt{k}")
                    xts.append(xt)
                for b in range(4):
                    raw = db.tile([128, N_DIM], F32, tag="xraw")
                    nc.sync.dma_start(
                        raw[:], X_d[bass.ds(c1 * P1C + b * 128, 128), :])
                    for k in range(2):
                        tp = ppB.tile([128, 128], F32, tag="ps")
                        nc.tensor.transpose(
                            tp[:], raw[:, 128 * k:128 * (k + 1)], ident[:])
                        nc.vector.tensor_copy(
                            xts[k][:, 128 * b:128 * (b + 1)], tp[:])
                for j in range(12):
                    ps = ppB.tile([128, P1C], F32, tag="ps")
                    nc.tensor.matmul(ps[:], bb_row[0:1, 128 * j:128 * (j + 1)],
                                     ones_row[0:1, 0:P1C],
                                     start=True, stop=False)
                    for k in range(2):
                        nc.tensor.matmul(ps[:], WihT[:, k * 12 + j, :],
                                         xts[k][:], start=False, stop=(k == 1))
                    gi_out = db.tile([128, P1C], F32, tag="giout")
                    nc.scalar.activation(gi_out[:], ps[:], AF.Copy)
                    nc.sync.dma_start(GiT_d[j, :, bass.ds(c1 * P1C, P1C)],
                                      gi_out[:])

            # ---- phase 2: the recurrence (+ fused output projection) ----
            Gi_v = GiT_d.rearrange("j p t -> p j t")

            def load_q(engines):
                return nc.values_load(
                    idx8[0:1, 0:1], engines=engines,
                    min_val=0, max_val=KA - 1,
                    skip_runtime_bounds_check=True)

            with tc.For_i(0, NCH * loops,
                          hint_engines=(PE, DVE, ACT, SP)) as ch_raw:
                ch = (ch_raw % NCH) if loops > 1 else ch_raw
                nc.sync.dma_start(giT[:], Gi_v[:, :, bass.ds(ch * CH, CH)])
                q = load_q([DVE, ACT, PE])

                for i in range(CH):
                    lg = ppL.tile([1, KA], F32, tag="lg")
                    # C_b bias (fp16 hi/lo pair) issues before gates finish;
                    # double-buffered PSUM lets it overlap prev step's argmax
                    nc.tensor.matmul(lg[:], one1h[:], Cb_hi[:],
                                     start=True, stop=False)
                    nc.tensor.matmul(lg[:], one1h[:], Cb_lo[:],
                                     start=False, stop=False)
                    if dummy_mm >= 2:
                        # fills the PE stall while the gates compute h_new
                        nc.tensor.matmul(dummy_ps[:], one1[:], ones_row[:],
                                         start=True, stop=True)
                    # gates: r on the critical path, z in its shadow
                    nc.vector.tensor_tensor(
                        g4r[:], giT[:, 0:4, bass.ds(i, 1)],
                        GHtab[:, 0:4, bass.ds(q, 1)], OP.add)
                    nc.vector.tensor_tensor(
                        g4z[:], giT[:, 4:8, bass.ds(i, 1)],
                        GHtab[:, 4:8, bass.ds(q, 1)], OP.add)
                    nc.scalar.activation(rr[:], g4r[:], AF.Sigmoid)
                    nc.vector.tensor_tensor(t4[:], rr[:],
                                            GHtab[:, 8:12, bass.ds(q, 1)],
                                            OP.mult)
                    nc.vector.tensor_tensor(u4[:], t4[:],
                                            giT[:, 8:12, bass.ds(i, 1)], OP.add)
                    nc.scalar.activation(zz[:], g4z[:], AF.Sigmoid)
                    nc.vector.tensor_tensor(omz[:], one4[:], zz[:], OP.subtract)
                    nc.vector.tensor_tensor(zm[:], zz[:],
                                            Mtab[:, :, bass.ds(q, 1)], OP.mult)
                    nc.scalar.activation(ng[:], u4[:], AF.Tanh)
                    # h_new = ng*(1-z) + z*h_prev
                    nc.vector.tensor_tensor(e4[:], ng[:], omz[:], OP.mult)
                    nc.vector.tensor_tensor(hnew[:], e4[:], zm[:], OP.add)
                    # logits += C_w @ h_new
                    for k in range(4):
                        nc.tensor.matmul(lg[:], hnew[:, k:k + 1],
                                         CwT[:, k, :],
                                         start=False, stop=(k == 3))
                    if dummy_mm >= 1:
                        # HAM warm-keeper: runs during the argmax window
                        nc.tensor.matmul(dummy_ps[:], one1[:], ones_row[:],
                                         start=True, stop=True)
                    if dummy_mm >= 3:
                        nc.tensor.matmul(dummy_ps[:], one1[:], ones_row[:],
                                         start=True, stop=True)
                    # argmax straight from PSUM
                    nc.vector.max(mx8[:], lg[:])
                    nc.vector.max_index(idx8[:], mx8[:], lg[:])
                    q2 = load_q([DVE, ACT, PE])
                    f = nc.values_load(wflag[0:1, bass.ds(q2, 1)],
                                       engines=[ACT, PE],
                                       skip_runtime_bounds_check=True)
                    with tc.If(f == 0):
                        # miss: write-once M/GHtab update (ACT+PE only)
                        nc.scalar.copy(Mtab[:, :, bass.ds(q2, 1)], hnew[:])
                        nc.scalar.copy(wflag[0:1, bass.ds(q2, 1)],
                                       POS[0:1, bass.ds(q2, 1)])
                        gh_matmuls(hnew)
                        nc.scalar.copy(GHtab[:, :, bass.ds(q2, 1)], gh_ps[:])
                    # h_out == Mtab[q2] on both paths (miss wrote it first)
                    nc.scalar.copy(Hbuf[:, i + 1, :],
                                   Mtab[:, :, bass.ds(q2, 1)])
                    q = q2

                    if (i + 1) % YB == 0:
                        # fused output projection for the last YB steps
                        tt = i // YB
                        ps_y = ppY.tile([YB, L_OUT], F32, tag="psy")
                        nc.tensor.matmul(ps_y[:], ones_row[0:1, 0:YB],
                                         Vb_row[:], start=True, stop=False)
                        for j in range(4):
                            nc.tensor.matmul(
                                ps_y[:],
                                Hbuf[:, 1 + tt * YB:1 + (tt + 1) * YB, j],
                                VwT[:, j, :], start=False, stop=(j == 3))
                        y16 = db.tile([YB, L_OUT], F16, tag="y16")
                        nc.vector.tensor_copy(y16[:], ps_y[:])
                        nc.sync.dma_start(
                            Y_d[bass.ds(ch * CH + tt * YB, YB), :], y16[:])

            if dummy_mm:
                # anchor so the warm-keeper matmuls aren't dead-code-eliminated
                junk = sc.tile([1, 512], F32, tag="junk")
                nc.vector.tensor_copy(junk[:], dummy_ps[:])
                nc.sync.dma_start(GiT_d[0, 0:1, 0:512], junk[:])

    nc.compile()
    return nc


_NC_CACHE = {}


def _get_nc(T=T_FULL, CH=128):
    key = (T, CH)
    if key not in _NC_CACHE:
        _NC_CACHE[key] = build_nc(T, CH)
    return _NC_CACHE[key]


def kernel(**inputs):
    nc = _get_nc()
    in_map = {k: np.ascontiguousarray(np.asarray(v, np.float32))
              for k, v in inputs.items()}
    res = bass_utils.run_bass_kernel_spmd(nc, [in_map], core_ids=[0])
    return res.results[0]["Y"].astype(np.float32)


# revision 12
# speedup vs baseline: 1.2439x; 1.0769x over previous
"""DeepMemoryMachine Trainium2 Bass kernel (v3).

Model: 16384-step sequential GRU + discrete write-once memory:
    h_new = GRU(h_prev, x_t)
    q     = argmax(C_w @ h_new + C_b)          (512 addresses)
    hit (written[q] & q>0):   h_out = M[q]     (read replaces state)
    miss:                     h_out = h_new;  first-visit q>0 writes M[q]=h_new

v3 changes over v2 (measured: exec wall 11.6s -> ~2.1s; bass build
130s -> ~6s; rel err 2.1e-4, trajectory exact vs the fp32 reference):
* 1-core execution. The axon tunnel moves ~45 MB/s; v2 replicated the
  22 MB inputs + 16 MB outputs x8 = ~9.5s of pure transfer.
* Program-size diet (CH=128 + phase-1 in a For_i + fused output
  projection): the XLA/neuronx compile inside the timed exec call
  scales with program size (~3s for v2's ~12k instructions).
* Y is emitted as float16 (halves output + donated-zeros traffic;
  ~2.4e-4 relative error, far under the 2e-2 gate).
* Output projection (Y = H @ V_w.T + V_b) fused into the recurrence
  chunk loop as per-128-step PE matmul blocks - no Ht HBM round-trip,
  no separate phase 3.
* DMA-transposed loads (4-byte descriptors) replaced by contiguous
  loads + PE transpose (f32 is not xbar-transposable).
* PE kept HAM-warm (2.4 GHz): logits PSUM double-buffered so step
  t+1's C_b bias matmul issues while step t's argmax reads the other
  bank; a dummy matmul fills the argmax window. v2 left PE idle
  >3.4us/step -> 1.2 GHz for every matmul (13.7us/step measured).
* Gate chain shortened: sigmoid split (r first, z off critical path),
  h_new = ng*(1-z) + z*Mq with both multiplicands precomputed in the
  tanh shadow; C_b bias matmul as an fp16 hi/lo pair (2x512 f16 rows
  instead of 1x512 f32-at-4-cycles).

Precision: min top-2 logit gap along the trajectory is 9.3e-6 abs, so
every matmul feeding h or the logits stays full fp32 (one flipped
argmax diverges the trajectory). fp16 only where exact-ish (C_b hi/lo
split, error ~1e-8) or after the recurrence (Y output, ~2.4e-4).

Layout: 512-vectors are SBUF [128, 4] with element (p, j) = v[p + 128*j];
1536-vectors are [128, 12] likewise.  Gi = X @ W_ih.T + b_ih + b_hh(rz)
is precomputed on-device into HBM as GiT[12, 128, T], streamed per chunk.
"""

import numpy as np

import concourse.bass as bass
import concourse.bacc as bacc
import concourse.mybir as mybir
import concourse.tile as tile
from concourse import bass_utils
from concourse.masks import make_identity

F32 = mybir.dt.float32
F16 = mybir.dt.float16
U32 = mybir.dt.uint32
DVE = mybir.EngineType.DVE
ACT = mybir.EngineType.Activation
PE = mybir.EngineType.PE
SP = mybir.EngineType.SP
AF = mybir.ActivationFunctionType
OP = mybir.AluOpType

T_FULL = 16384
N_DIM = 256
M_DIM = 512     # hidden size; [128, 4] layout
KA = 512        # number of addresses (K+1)
L_OUT = 256
G3 = 3 * M_DIM  # 1536; [128, 12] layout


def build_nc(T=T_FULL, CH=128, loops=1, dummy_mm=0):
    YB = min(CH, 128)
    assert T % CH == 0 and CH % YB == 0 and T % 512 == 0
    NCH = T // CH
    nc = bacc.Bacc("TRN2", target_bir_lowering=False, debug=False,
                   enable_asserts=False)

    X_d = nc.dram_tensor("X", [T, N_DIM], F32, kind="ExternalInput")
    h0_d = nc.dram_tensor("h0", [M_DIM], F32, kind="ExternalInput")
    Wih_d = nc.dram_tensor("W_ih", [G3, N_DIM], F32, kind="ExternalInput")
    Whh_d = nc.dram_tensor("W_hh", [G3, M_DIM], F32, kind="ExternalInput")
    bih_d = nc.dram_tensor("b_ih", [G3], F32, kind="ExternalInput")
    bhh_d = nc.dram_tensor("b_hh", [G3], F32, kind="ExternalInput")
    Cw_d = nc.dram_tensor("C_w", [KA, M_DIM], F32, kind="ExternalInput")
    Cb_d = nc.dram_tensor("C_b", [KA], F32, kind="ExternalInput")
    Vw_d = nc.dram_tensor("V_w", [L_OUT, M_DIM], F32, kind="ExternalInput")
    Vb_d = nc.dram_tensor("V_b", [L_OUT], F32, kind="ExternalInput")
    Y_d = nc.dram_tensor("Y", [T, L_OUT], F16, kind="ExternalOutput")

    GiT_d = nc.dram_tensor("GiT", [12, 128, T], F32, kind="Internal")

    with tile.TileContext(nc) as tc:
        with (
            tc.tile_pool(name="state", bufs=1) as st,
            tc.tile_pool(name="scratch", bufs=1) as sc,
            tc.tile_pool(name="dbuf", bufs=2) as db,
            tc.tile_pool(name="psA", bufs=1, space="PSUM") as ppA,
            tc.tile_pool(name="psB", bufs=2, space="PSUM") as ppB,
            tc.tile_pool(name="psL", bufs=2, space="PSUM") as ppL,
            tc.tile_pool(name="psY", bufs=1, space="PSUM") as ppY,
        ):
            # ---- persistent weights / state in SBUF ----
            WihT = st.tile([128, 24, 128], F32)      # (kp, k*12+j, m)
            WhhT = st.tile([128, 48, 128], F32)      # (dp, k*12+j, m)
            CwT = st.tile([128, 4, KA], F32)         # (dp, k, a)
            VwT = st.tile([128, 4, L_OUT], F32)      # (dp, k, l)
            bb_row = st.tile([1, G3], F32)           # b_ih + b_hh (r,z only)
            bhh_row = st.tile([1, G3], F32)
            bhh_nT = st.tile([1, M_DIM], F32)        # b_hn row (for GHtab)
            Cb_hi = st.tile([1, KA], F16)
            Cb_lo = st.tile([1, KA], F16)
            Vb_row = st.tile([1, L_OUT], F32)
            ones_row = st.tile([1, 512], F32)
            one4 = st.tile([128, 4], F32)
            one1 = st.tile([1, 1], F32)
            one1h = st.tile([1, 1], F16)
            POS = st.tile([1, KA], U32)              # [0,1,1,...,1]
            ident = st.tile([128, 128], F32)         # PE transpose identity

            GHtab = st.tile([128, 12, KA], F32)      # memoized W_hh@M[q] + bhn
            Mtab = st.tile([128, 4, KA], F32)        # memory rows
            wflag = st.tile([1, KA], U32)            # written flags
            Hbuf = st.tile([128, CH + 1, 4], F32)    # h_out history (chunk)
            giT = st.tile([128, 12, CH], F32)        # streamed Gi chunk
            idx8 = st.tile([1, 8], U32)
            mx8 = st.tile([1, 8], F32)

            g8 = sc.tile([128, 8], F32)
            rz = sc.tile([128, 8], F32)
            d4 = sc.tile([128, 4], F32)
            t4 = sc.tile([128, 4], F32)
            u4 = sc.tile([128, 4], F32)
            ng = sc.tile([128, 4], F32)
            e4 = sc.tile([128, 4], F32)
            hnew = sc.tile([128, 4], F32)

            gh_ps = ppA.tile([128, 12], F32)
            dummy_ps = ppA.tile([1, 512], F32)

            # ---- constants ----
            nc.vector.memset(ones_row[:], 1.0)
            nc.vector.memset(one4[:], 1.0)
            nc.vector.memset(one1[:], 1.0)
            nc.vector.memset(one1h[:], 1.0)
            nc.vector.memset(wflag[:], 0)
            nc.vector.memset(POS[:], 1)
            nc.vector.memset(POS[0:1, 0:1], 0)
            nc.vector.memset(idx8[:], 0)
            nc.vector.memset(Mtab[:], 0.0)
            nc.vector.memset(GHtab[:], 0.0)
            nc.vector.memset(Hbuf[:], 0.0)
            make_identity(nc, ident[:])

            # ---- one-time weight loads: contiguous DMA + PE transpose ----
            def load_T(dst_ap, src_ap, tag):
                """dst[128,128] (SBUF) = src[128,128] (DRAM).T via PE."""
                raw = db.tile([128, 128], F32, tag=f"ldT_{tag}")
                nc.sync.dma_start(raw[:], src_ap)
                tp = ppB.tile([128, 128], F32, tag="ps")
                nc.tensor.transpose(tp[:], raw[:], ident[:])
                nc.vector.tensor_copy(dst_ap, tp[:])

            for j in range(12):
                for k in range(2):
                    load_T(WihT[:, k * 12 + j, :],
                           Wih_d[128 * j:128 * (j + 1),
                                 128 * k:128 * (k + 1)], "w")
                for k in range(4):
                    load_T(WhhT[:, k * 12 + j, :],
                           Whh_d[128 * j:128 * (j + 1),
                                 128 * k:128 * (k + 1)], "w")
            for k in range(4):
                for a in range(4):
                    load_T(CwT[:, k, 128 * a:128 * (a + 1)],
                           Cw_d[128 * a:128 * (a + 1),
                                128 * k:128 * (k + 1)], "w")
                for a in range(2):
                    load_T(VwT[:, k, 128 * a:128 * (a + 1)],
                           Vw_d[128 * a:128 * (a + 1),
                                128 * k:128 * (k + 1)], "w")

            nc.sync.dma_start(bb_row[:], bih_d[None, :])
            nc.sync.dma_start(bhh_row[:], bhh_d[None, :])
            nc.sync.dma_start(Vb_row[:], Vb_d[None, :])
            nc.sync.dma_start(bhh_nT[:], bhh_d[None, 2 * M_DIM:3 * M_DIM])
            # fold b_hh into Gi for the r,z gates only; b_hn lives in GHtab
            # (torch GRU: n = tanh(W_in x + b_in + r*(W_hn h + b_hn)))
            nc.vector.tensor_tensor(bb_row[0:1, 0:2 * M_DIM],
                                    bb_row[0:1, 0:2 * M_DIM],
                                    bhh_row[0:1, 0:2 * M_DIM], OP.add)

            # C_b as an fp16 hi/lo pair (residual error ~1e-8)
            Cb_f32 = sc.tile([1, KA], F32, tag="cbtmp")
            Cb_rest = sc.tile([1, KA], F32, tag="cbtmp2")
            nc.sync.dma_start(Cb_f32[:], Cb_d[None, :])
            nc.vector.tensor_copy(Cb_hi[:], Cb_f32[:])       # f32 -> f16 round
            nc.vector.tensor_copy(Cb_rest[:], Cb_hi[:])      # f16 -> f32 exact
            nc.vector.tensor_tensor(Cb_rest[:], Cb_f32[:], Cb_rest[:],
                                    OP.subtract)
            nc.vector.tensor_copy(Cb_lo[:], Cb_rest[:])      # residual -> f16

            # h_prev is always read from Mtab[q]; q starts at 0
            nc.sync.dma_start(Mtab[:, :, 0],
                              h0_d.rearrange("(j p) -> p j", p=128))
            h0_sb = sc.tile([128, 4], F32, tag="h0sb")
            nc.sync.dma_start(h0_sb[:], h0_d.rearrange("(j p) -> p j", p=128))

            def gh_matmuls(hsrc):
                """gh_ps[:, j] = (W_hh @ h)[128j:128j+128], + b_hn on j>=8."""
                for j in range(12):
                    for k in range(4):
                        nc.tensor.matmul(gh_ps[:, j:j + 1],
                                         WhhT[:, k * 12 + j, :],
                                         hsrc[:, k:k + 1],
                                         start=(k == 0),
                                         stop=(k == 3 and j < 8))
                    if j >= 8:
                        nc.tensor.matmul(gh_ps[:, j:j + 1],
                                         bhh_nT[0:1, 128 * (j - 8):128 * (j - 7)],
                                         one1[:], start=False, stop=True)

            gh_matmuls(h0_sb)
            nc.vector.tensor_copy(GHtab[:, :, 0], gh_ps[:])

            # ---- phase 1: GiT = (X @ W_ih.T + bb) as [12,128,T] ----
            P1C = 512
            with tc.For_i(0, T // P1C) as c1:
                xts = []
                for k in range(2):
                    xt = db.tile([128, P1C], F32, tag=f"xt{k}")
                    xts.append(xt)
                for b in range(4):
                    raw = db.tile([128, N_DIM], F32, tag="xraw")
                    nc.sync.dma_start(
                        raw[:], X_d[bass.ds(c1 * P1C + b * 128, 128), :])
                    for k in range(2):
                        tp = ppB.tile([128, 128], F32, tag="ps")
                        nc.tensor.transpose(
                            tp[:], raw[:, 128 * k:128 * (k + 1)], ident[:])
                        nc.vector.tensor_copy(
                            xts[k][:, 128 * b:128 * (b + 1)], tp[:])
                for j in range(12):
                    ps = ppB.tile([128, P1C], F32, tag="ps")
                    nc.tensor.matmul(ps[:], bb_row[0:1, 128 * j:128 * (j + 1)],
                                     ones_row[0:1, 0:P1C],
                                     start=True, stop=False)
                    for k in range(2):
                        nc.tensor.matmul(ps[:], WihT[:, k * 12 + j, :],
                                         xts[k][:], start=False, stop=(k == 1))
                    gi_out = db.tile([128, P1C], F32, tag="giout")
                    nc.scalar.activation(gi_out[:], ps[:], AF.Copy)
                    nc.sync.dma_start(GiT_d[j, :, bass.ds(c1 * P1C, P1C)],
                                      gi_out[:])

            # ---- phase 2: the recurrence (+ fused output projection) ----
            Gi_v = GiT_d.rearrange("j p t -> p j t")

            def load_q(engines):
                return nc.values_load(
                    idx8[0:1, 0:1], engines=engines,
                    min_val=0, max_val=KA - 1,
                    skip_runtime_bounds_check=True)

            with tc.For_i(0, NCH * loops,
                          hint_engines=(PE, DVE, ACT, SP)) as ch_raw:
                ch = (ch_raw % NCH) if loops > 1 else ch_raw
                nc.sync.dma_start(giT[:], Gi_v[:, :, bass.ds(ch * CH, CH)])
                q = load_q([DVE, ACT, PE])

                for i in range(CH):
                    lg = ppL.tile([1, KA], F32, tag="lg")
                    # C_b bias (fp16 hi/lo pair) issues before gates finish;
                    # double-buffered PSUM lets it overlap prev step's argmax
                    nc.tensor.matmul(lg[:], one1h[:], Cb_hi[:],
                                     start=True, stop=False)
                    nc.tensor.matmul(lg[:], one1h[:], Cb_lo[:],
                                     start=False, stop=False)
                    if dummy_mm >= 2:
                        # fills the PE stall while the gates compute h_new
                        nc.tensor.matmul(dummy_ps[:], one1[:], ones_row[:],
                                         start=True, stop=True)
                    # gates (v2-verified arithmetic, minimal op count)
                    nc.vector.tensor_tensor(
                        g8[:], giT[:, 0:8, bass.ds(i, 1)],
                        GHtab[:, 0:8, bass.ds(q, 1)], OP.add)
                    nc.scalar.activation(rz[:], g8[:], AF.Sigmoid)
                    nc.vector.tensor_tensor(t4[:], rz[:, 0:4],
                                            GHtab[:, 8:12, bass.ds(q, 1)],
                                            OP.mult)
                    nc.vector.tensor_tensor(u4[:], t4[:],
                                            giT[:, 8:12, bass.ds(i, 1)], OP.add)
                    nc.scalar.activation(ng[:], u4[:], AF.Tanh)
                    # h_new = ng + z*(h_prev - ng)
                    nc.vector.tensor_tensor(d4[:], Mtab[:, :, bass.ds(q, 1)],
                                            ng[:], OP.subtract)
                    nc.vector.tensor_tensor(e4[:], rz[:, 4:8], d4[:], OP.mult)
                    nc.vector.tensor_tensor(hnew[:], ng[:], e4[:], OP.add)
                    # logits += C_w @ h_new
                    for k in range(4):
                        nc.tensor.matmul(lg[:], hnew[:, k:k + 1],
                                         CwT[:, k, :],
                                         start=False, stop=(k == 3))
                    if dummy_mm >= 1:
                        # HAM warm-keeper: runs during the argmax window
                        nc.tensor.matmul(dummy_ps[:], one1[:], ones_row[:],
                                         start=True, stop=True)
                    if dummy_mm >= 3:
                        nc.tensor.matmul(dummy_ps[:], one1[:], ones_row[:],
                                         start=True, stop=True)
                    # argmax straight from PSUM
                    nc.vector.max(mx8[:], lg[:])
                    nc.vector.max_index(idx8[:], mx8[:], lg[:])
                    q2 = load_q([DVE, ACT, PE])
                    f = nc.values_load(wflag[0:1, bass.ds(q2, 1)],
                                       engines=[ACT, PE],
                                       skip_runtime_bounds_check=True)
                    with tc.If(f == 0):
                        # miss: write-once M/GHtab update (ACT+PE only)
                        nc.scalar.copy(Mtab[:, :, bass.ds(q2, 1)], hnew[:])
                        nc.scalar.copy(wflag[0:1, bass.ds(q2, 1)],
                                       POS[0:1, bass.ds(q2, 1)])
                        gh_matmuls(hnew)
                        nc.scalar.copy(GHtab[:, :, bass.ds(q2, 1)], gh_ps[:])
                    # h_out == Mtab[q2] on both paths (miss wrote it first)
                    nc.scalar.copy(Hbuf[:, i + 1, :],
                                   Mtab[:, :, bass.ds(q2, 1)])
                    q = q2

                    if (i + 1) % YB == 0:
                        # fused output projection for the last YB steps
                        tt = i // YB
                        ps_y = ppY.tile([YB, L_OUT], F32, tag="psy")
                        nc.tensor.matmul(ps_y[:], ones_row[0:1, 0:YB],
                                         Vb_row[:], start=True, stop=False)
                        for j in range(4):
                            nc.tensor.matmul(
                                ps_y[:],
                                Hbuf[:, 1 + tt * YB:1 + (tt + 1) * YB, j],
                                VwT[:, j, :], start=False, stop=(j == 3))
                        y16 = db.tile([YB, L_OUT], F16, tag="y16")
                        nc.vector.tensor_copy(y16[:], ps_y[:])
                        nc.sync.dma_start(
                            Y_d[bass.ds(ch * CH + tt * YB, YB), :], y16[:])

            if dummy_mm:
                # anchor so the warm-keeper matmuls aren't dead-code-eliminated
                junk = sc.tile([1, 512], F32, tag="junk")
                nc.vector.tensor_copy(junk[:], dummy_ps[:])
                nc.sync.dma_start(GiT_d[0, 0:1, 0:512], junk[:])

    nc.compile()
    return nc


_NC_CACHE = {}


def _get_nc(T=T_FULL, CH=128):
    key = (T, CH)
    if key not in _NC_CACHE:
        _NC_CACHE[key] = build_nc(T, CH)
    return _NC_CACHE[key]


def kernel(**inputs):
    nc = _get_nc()
    in_map = {k: np.ascontiguousarray(np.asarray(v, np.float32))
              for k, v in inputs.items()}
    res = bass_utils.run_bass_kernel_spmd(nc, [in_map], core_ids=[0])
    return res.results[0]["Y"].astype(np.float32)


# revision 13
# speedup vs baseline: 1.8174x; 1.4611x over previous
"""DeepMemoryMachine Trainium2 Bass kernel (v3).

Model: 16384-step sequential GRU + discrete write-once memory:
    h_new = GRU(h_prev, x_t)
    q     = argmax(C_w @ h_new + C_b)          (512 addresses)
    hit (written[q] & q>0):   h_out = M[q]     (read replaces state)
    miss:                     h_out = h_new;  first-visit q>0 writes M[q]=h_new

v3 changes over v2 (measured: exec wall 11.6s -> 2.7s first call /
1.73s warm; bass build
130s -> ~6s; rel err 2.1e-4, trajectory exact vs the fp32 reference):
* 1-core execution. The axon tunnel moves ~45 MB/s; v2 replicated the
  22 MB inputs + 16 MB outputs x8 = ~9.5s of pure transfer.
* Program-size diet (CH=128 + phase-1 in a For_i + fused output
  projection): the XLA/neuronx compile inside the timed exec call
  scales with program size (~3s for v2's ~12k instructions).
* Y is emitted as float16 (halves output + donated-zeros traffic;
  ~2.4e-4 relative error, far under the 2e-2 gate).
* Output projection (Y = H @ V_w.T + V_b) fused into the recurrence
  chunk loop as per-128-step PE matmul blocks - no Ht HBM round-trip,
  no separate phase 3.
* DMA-transposed loads (4-byte descriptors) replaced by contiguous
  loads + PE transpose (f32 is not xbar-transposable).
* Logits PSUM double-buffered so step t+1's C_b bias matmul issues
  while step t's argmax reads the other bank; C_b bias as an fp16
  hi/lo pair (2x512 f16 rows instead of 1x512 f32-at-4-cycles).
  (HAM warm-keeper dummy matmuls measured neutral - the per-step If
  branch stalls PE's in-order queue regardless - and are off by
  default; the per-step tc.If costs ~7us/step but branch-free
  restructures measured slower, see the session notes.)

Precision: min top-2 logit gap along the trajectory is 9.3e-6 abs, so
every matmul feeding h or the logits stays full fp32 (one flipped
argmax diverges the trajectory). fp16 only where exact-ish (C_b hi/lo
split, error ~1e-8) or after the recurrence (Y output, ~2.4e-4).

Layout: 512-vectors are SBUF [128, 4] with element (p, j) = v[p + 128*j];
1536-vectors are [128, 12] likewise.  Gi = X @ W_ih.T + b_ih + b_hh(rz)
is precomputed on-device into HBM as GiT[12, 128, T], streamed per chunk.
"""

import numpy as np

import concourse.bass as bass
import concourse.bacc as bacc
import concourse.mybir as mybir
import concourse.tile as tile
from concourse import bass_utils
from concourse.masks import make_identity

F32 = mybir.dt.float32
F16 = mybir.dt.float16
U32 = mybir.dt.uint32
DVE = mybir.EngineType.DVE
ACT = mybir.EngineType.Activation
PE = mybir.EngineType.PE
SP = mybir.EngineType.SP
AF = mybir.ActivationFunctionType
OP = mybir.AluOpType

T_FULL = 16384
N_DIM = 256
M_DIM = 512     # hidden size; [128, 4] layout
KA = 512        # number of addresses (K+1)
L_OUT = 256
G3 = 3 * M_DIM  # 1536; [128, 12] layout


def build_nc(T=T_FULL, CH=128, loops=1, dummy_mm=0):
    YB = min(CH, 128)
    assert T % CH == 0 and CH % YB == 0 and T % 512 == 0
    NCH = T // CH
    nc = bacc.Bacc("TRN2", target_bir_lowering=False, debug=False,
                   enable_asserts=False)

    X_d = nc.dram_tensor("X", [T, N_DIM], F32, kind="ExternalInput")
    h0_d = nc.dram_tensor("h0", [M_DIM], F32, kind="ExternalInput")
    Wih_d = nc.dram_tensor("W_ih", [G3, N_DIM], F32, kind="ExternalInput")
    Whh_d = nc.dram_tensor("W_hh", [G3, M_DIM], F32, kind="ExternalInput")
    bih_d = nc.dram_tensor("b_ih", [G3], F32, kind="ExternalInput")
    bhh_d = nc.dram_tensor("b_hh", [G3], F32, kind="ExternalInput")
    Cw_d = nc.dram_tensor("C_w", [KA, M_DIM], F32, kind="ExternalInput")
    Cb_d = nc.dram_tensor("C_b", [KA], F32, kind="ExternalInput")
    Vw_d = nc.dram_tensor("V_w", [L_OUT, M_DIM], F32, kind="ExternalInput")
    Vb_d = nc.dram_tensor("V_b", [L_OUT], F32, kind="ExternalInput")
    Y_d = nc.dram_tensor("Y", [T, L_OUT], F16, kind="ExternalOutput")

    GiT_d = nc.dram_tensor("GiT", [12, 128, T], F32, kind="Internal")

    with tile.TileContext(nc) as tc:
        with (
            tc.tile_pool(name="state", bufs=1) as st,
            tc.tile_pool(name="scratch", bufs=1) as sc,
            tc.tile_pool(name="dbuf", bufs=2) as db,
            tc.tile_pool(name="psA", bufs=1, space="PSUM") as ppA,
            tc.tile_pool(name="psB", bufs=2, space="PSUM") as ppB,
            tc.tile_pool(name="psL", bufs=2, space="PSUM") as ppL,
            tc.tile_pool(name="psY", bufs=1, space="PSUM") as ppY,
        ):
            # ---- persistent weights / state in SBUF ----
            WihT = st.tile([128, 24, 128], F32)      # (kp, k*12+j, m)
            WhhT = st.tile([128, 48, 128], F32)      # (dp, k*12+j, m)
            CwT = st.tile([128, 4, KA], F32)         # (dp, k, a)
            VwT = st.tile([128, 4, L_OUT], F32)      # (dp, k, l)
            bb_row = st.tile([1, G3], F32)           # b_ih + b_hh (r,z only)
            bhh_row = st.tile([1, G3], F32)
            bhh_nT = st.tile([1, M_DIM], F32)        # b_hn row (for GHtab)
            Cb_hi = st.tile([1, KA], F16)
            Cb_lo = st.tile([1, KA], F16)
            Vb_row = st.tile([1, L_OUT], F32)
            ones_row = st.tile([1, 512], F32)
            one4 = st.tile([128, 4], F32)
            one1 = st.tile([1, 1], F32)
            one1h = st.tile([1, 1], F16)
            POS = st.tile([1, KA], U32)              # [0,1,1,...,1]
            ident = st.tile([128, 128], F32)         # PE transpose identity

            GHtab = st.tile([128, 12, KA], F32)      # memoized W_hh@M[q] + bhn
            Mtab = st.tile([128, 4, KA], F32)        # memory rows
            wflag = st.tile([1, KA], U32)            # written flags
            Hbuf = st.tile([128, CH + 1, 4], F32)    # h_out history (chunk)
            giT = st.tile([128, 12, CH], F32)        # streamed Gi chunk
            idx8 = st.tile([1, 8], U32)
            mx8 = st.tile([1, 8], F32)

            g8 = sc.tile([128, 8], F32)
            rz = sc.tile([128, 8], F32)
            d4 = sc.tile([128, 4], F32)
            t4 = sc.tile([128, 4], F32)
            u4 = sc.tile([128, 4], F32)
            ng = sc.tile([128, 4], F32)
            e4 = sc.tile([128, 4], F32)
            hnew = sc.tile([128, 4], F32)

            gh_ps = ppA.tile([128, 12], F32)
            dummy_ps = ppA.tile([1, 512], F32)

            # ---- constants ----
            nc.vector.memset(ones_row[:], 1.0)
            nc.vector.memset(one4[:], 1.0)
            nc.vector.memset(one1[:], 1.0)
            nc.vector.memset(one1h[:], 1.0)
            nc.vector.memset(wflag[:], 0)
            nc.vector.memset(POS[:], 1)
            nc.vector.memset(POS[0:1, 0:1], 0)
            nc.vector.memset(idx8[:], 0)
            nc.vector.memset(Mtab[:], 0.0)
            nc.vector.memset(GHtab[:], 0.0)
            nc.vector.memset(Hbuf[:], 0.0)
            make_identity(nc, ident[:])

            # ---- one-time weight loads: contiguous DMA + PE transpose ----
            def load_T(dst_ap, src_ap, tag):
                """dst[128,128] (SBUF) = src[128,128] (DRAM).T via PE."""
                raw = db.tile([128, 128], F32, tag=f"ldT_{tag}")
                nc.sync.dma_start(raw[:], src_ap)
                tp = ppB.tile([128, 128], F32, tag="ps")
                nc.tensor.transpose(tp[:], raw[:], ident[:])
                nc.vector.tensor_copy(dst_ap, tp[:])

            for j in range(12):
                for k in range(2):
                    load_T(WihT[:, k * 12 + j, :],
                           Wih_d[128 * j:128 * (j + 1),
                                 128 * k:128 * (k + 1)], "w")
                for k in range(4):
                    load_T(WhhT[:, k * 12 + j, :],
                           Whh_d[128 * j:128 * (j + 1),
                                 128 * k:128 * (k + 1)], "w")
            for k in range(4):
                for a in range(4):
                    load_T(CwT[:, k, 128 * a:128 * (a + 1)],
                           Cw_d[128 * a:128 * (a + 1),
                                128 * k:128 * (k + 1)], "w")
                for a in range(2):
                    load_T(VwT[:, k, 128 * a:128 * (a + 1)],
                           Vw_d[128 * a:128 * (a + 1),
                                128 * k:128 * (k + 1)], "w")

            nc.sync.dma_start(bb_row[:], bih_d[None, :])
            nc.sync.dma_start(bhh_row[:], bhh_d[None, :])
            nc.sync.dma_start(Vb_row[:], Vb_d[None, :])
            nc.sync.dma_start(bhh_nT[:], bhh_d[None, 2 * M_DIM:3 * M_DIM])
            # fold b_hh into Gi for the r,z gates only; b_hn lives in GHtab
            # (torch GRU: n = tanh(W_in x + b_in + r*(W_hn h + b_hn)))
            nc.vector.tensor_tensor(bb_row[0:1, 0:2 * M_DIM],
                                    bb_row[0:1, 0:2 * M_DIM],
                                    bhh_row[0:1, 0:2 * M_DIM], OP.add)

            # C_b as an fp16 hi/lo pair (residual error ~1e-8)
            Cb_f32 = sc.tile([1, KA], F32, tag="cbtmp")
            Cb_rest = sc.tile([1, KA], F32, tag="cbtmp2")
            nc.sync.dma_start(Cb_f32[:], Cb_d[None, :])
            nc.vector.tensor_copy(Cb_hi[:], Cb_f32[:])       # f32 -> f16 round
            nc.vector.tensor_copy(Cb_rest[:], Cb_hi[:])      # f16 -> f32 exact
            nc.vector.tensor_tensor(Cb_rest[:], Cb_f32[:], Cb_rest[:],
                                    OP.subtract)
            nc.vector.tensor_copy(Cb_lo[:], Cb_rest[:])      # residual -> f16

            # h_prev is always read from Mtab[q]; q starts at 0
            nc.sync.dma_start(Mtab[:, :, 0],
                              h0_d.rearrange("(j p) -> p j", p=128))
            h0_sb = sc.tile([128, 4], F32, tag="h0sb")
            nc.sync.dma_start(h0_sb[:], h0_d.rearrange("(j p) -> p j", p=128))

            def gh_matmuls(hsrc):
                """gh_ps[:, j] = (W_hh @ h)[128j:128j+128], + b_hn on j>=8."""
                for j in range(12):
                    for k in range(4):
                        nc.tensor.matmul(gh_ps[:, j:j + 1],
                                         WhhT[:, k * 12 + j, :],
                                         hsrc[:, k:k + 1],
                                         start=(k == 0),
                                         stop=(k == 3 and j < 8))
                    if j >= 8:
                        nc.tensor.matmul(gh_ps[:, j:j + 1],
                                         bhh_nT[0:1, 128 * (j - 8):128 * (j - 7)],
                                         one1[:], start=False, stop=True)

            gh_matmuls(h0_sb)
            nc.vector.tensor_copy(GHtab[:, :, 0], gh_ps[:])

            # ---- phase 1: GiT = (X @ W_ih.T + bb) as [12,128,T] ----
            P1C = 512
            with tc.For_i(0, T // P1C) as c1:
                xts = []
                for k in range(2):
                    xt = db.tile([128, P1C], F32, tag=f"xt{k}")
                    xts.append(xt)
                for b in range(4):
                    raw = db.tile([128, N_DIM], F32, tag="xraw")
                    nc.sync.dma_start(
                        raw[:], X_d[bass.ds(c1 * P1C + b * 128, 128), :])
                    for k in range(2):
                        tp = ppB.tile([128, 128], F32, tag="ps")
                        nc.tensor.transpose(
                            tp[:], raw[:, 128 * k:128 * (k + 1)], ident[:])
                        nc.vector.tensor_copy(
                            xts[k][:, 128 * b:128 * (b + 1)], tp[:])
                for j in range(12):
                    ps = ppB.tile([128, P1C], F32, tag="ps")
                    nc.tensor.matmul(ps[:], bb_row[0:1, 128 * j:128 * (j + 1)],
                                     ones_row[0:1, 0:P1C],
                                     start=True, stop=False)
                    for k in range(2):
                        nc.tensor.matmul(ps[:], WihT[:, k * 12 + j, :],
                                         xts[k][:], start=False, stop=(k == 1))
                    gi_out = db.tile([128, P1C], F32, tag="giout")
                    nc.scalar.activation(gi_out[:], ps[:], AF.Copy)
                    nc.sync.dma_start(GiT_d[j, :, bass.ds(c1 * P1C, P1C)],
                                      gi_out[:])

            # ---- phase 2: the recurrence (+ fused output projection) ----
            Gi_v = GiT_d.rearrange("j p t -> p j t")

            def load_q(engines):
                return nc.values_load(
                    idx8[0:1, 0:1], engines=engines,
                    min_val=0, max_val=KA - 1,
                    skip_runtime_bounds_check=True)

            with tc.For_i(0, NCH * loops,
                          hint_engines=(PE, DVE, ACT, SP)) as ch_raw:
                ch = (ch_raw % NCH) if loops > 1 else ch_raw
                nc.sync.dma_start(giT[:], Gi_v[:, :, bass.ds(ch * CH, CH)])
                q = load_q([DVE, ACT, PE])

                for i in range(CH):
                    lg = ppL.tile([1, KA], F32, tag="lg")
                    # C_b bias (fp16 hi/lo pair) issues before gates finish;
                    # double-buffered PSUM lets it overlap prev step's argmax
                    nc.tensor.matmul(lg[:], one1h[:], Cb_hi[:],
                                     start=True, stop=False)
                    nc.tensor.matmul(lg[:], one1h[:], Cb_lo[:],
                                     start=False, stop=False)
                    if dummy_mm >= 2:
                        # fills the PE stall while the gates compute h_new
                        nc.tensor.matmul(dummy_ps[:], one1[:], ones_row[:],
                                         start=True, stop=True)
                    # gates (v2-verified arithmetic, minimal op count)
                    nc.vector.tensor_tensor(
                        g8[:], giT[:, 0:8, bass.ds(i, 1)],
                        GHtab[:, 0:8, bass.ds(q, 1)], OP.add)
                    nc.scalar.activation(rz[:], g8[:], AF.Sigmoid)
                    nc.vector.tensor_tensor(t4[:], rz[:, 0:4],
                                            GHtab[:, 8:12, bass.ds(q, 1)],
                                            OP.mult)
                    nc.vector.tensor_tensor(u4[:], t4[:],
                                            giT[:, 8:12, bass.ds(i, 1)], OP.add)
                    nc.scalar.activation(ng[:], u4[:], AF.Tanh)
                    # h_new = ng + z*(h_prev - ng)
                    nc.vector.tensor_tensor(d4[:], Mtab[:, :, bass.ds(q, 1)],
                                            ng[:], OP.subtract)
                    nc.vector.tensor_tensor(e4[:], rz[:, 4:8], d4[:], OP.mult)
                    nc.vector.tensor_tensor(hnew[:], ng[:], e4[:], OP.add)
                    # logits += C_w @ h_new
                    for k in range(4):
                        nc.tensor.matmul(lg[:], hnew[:, k:k + 1],
                                         CwT[:, k, :],
                                         start=False, stop=(k == 3))
                    if dummy_mm >= 1:
                        # HAM warm-keeper: runs during the argmax window
                        nc.tensor.matmul(dummy_ps[:], one1[:], ones_row[:],
                                         start=True, stop=True)
                    if dummy_mm >= 3:
                        nc.tensor.matmul(dummy_ps[:], one1[:], ones_row[:],
                                         start=True, stop=True)
                    # argmax straight from PSUM
                    nc.vector.max(mx8[:], lg[:])
                    nc.vector.max_index(idx8[:], mx8[:], lg[:])
                    q2 = load_q([DVE, ACT, PE])
                    f = nc.values_load(wflag[0:1, bass.ds(q2, 1)],
                                       engines=[ACT, PE],
                                       skip_runtime_bounds_check=True)
                    with tc.If(f == 0):
                        # miss: write-once M/GHtab update (ACT+PE only)
                        nc.scalar.copy(Mtab[:, :, bass.ds(q2, 1)], hnew[:])
                        nc.scalar.copy(wflag[0:1, bass.ds(q2, 1)],
                                       POS[0:1, bass.ds(q2, 1)])
                        gh_matmuls(hnew)
                        nc.scalar.copy(GHtab[:, :, bass.ds(q2, 1)], gh_ps[:])
                    # h_out == Mtab[q2] on both paths (miss wrote it first)
                    nc.scalar.copy(Hbuf[:, i + 1, :],
                                   Mtab[:, :, bass.ds(q2, 1)])
                    q = q2

                    if (i + 1) % YB == 0:
                        # fused output projection for the last YB steps
                        tt = i // YB
                        ps_y = ppY.tile([YB, L_OUT], F32, tag="psy")
                        nc.tensor.matmul(ps_y[:], ones_row[0:1, 0:YB],
                                         Vb_row[:], start=True, stop=False)
                        for j in range(4):
                            nc.tensor.matmul(
                                ps_y[:],
                                Hbuf[:, 1 + tt * YB:1 + (tt + 1) * YB, j],
                                VwT[:, j, :], start=False, stop=(j == 3))
                        y16 = db.tile([YB, L_OUT], F16, tag="y16")
                        nc.vector.tensor_copy(y16[:], ps_y[:])
                        nc.sync.dma_start(
                            Y_d[bass.ds(ch * CH + tt * YB, YB), :], y16[:])

            if dummy_mm:
                # anchor so the warm-keeper matmuls aren't dead-code-eliminated
                junk = sc.tile([1, 512], F32, tag="junk")
                nc.vector.tensor_copy(junk[:], dummy_ps[:])
                nc.sync.dma_start(GiT_d[0, 0:1, 0:512], junk[:])

    nc.compile()
    return nc


_NC_CACHE = {}


def _get_nc(T=T_FULL, CH=128):
    key = (T, CH)
    if key not in _NC_CACHE:
        _NC_CACHE[key] = build_nc(T, CH)
    return _NC_CACHE[key]


def kernel(**inputs):
    nc = _get_nc()
    in_map = {k: np.ascontiguousarray(np.asarray(v, np.float32))
              for k, v in inputs.items()}
    res = bass_utils.run_bass_kernel_spmd(nc, [in_map], core_ids=[0])
    return res.results[0]["Y"].astype(np.float32)
